# revision 16
# baseline (speedup 1.0000x reference)
"""Epipolar attention kernel for Trainium2 (8 NeuronCores, batch-parallel).

Math notes (derived from the reference):
  - f_tar is dead code: the output only depends on f_src / K1 / K2 / R / t.
  - With x0=0, x1=W the distance field factorizes rank-3:
        d[b,i,j] = |px_i*alpha[b,j] + py_i*beta[b,j] + gamma[b,j]|
    where alpha = dy/L, beta = -dx/L, gamma = y0*dx/L, L = sqrt(dx^2+dy^2).
  - softmax_j(5*(d-0.1)) == softmax_j(5*d)           (shift invariance)
  - softmax_i(1 - p)     == softmax_i(-p), and p in (0,1] means exp(-p) needs
    no max subtraction.
The 3x3 SVD / inverse chain (B=16) plus the rank-3 coefficient prep is O(B*HW)
host work; all O(B*HW^2) work runs on the NeuronCores.

v4 performance structure:
  - Stage-1 S = P^T Q runs as K=6 matmuls (hi+lo bf16 rows merged; K=6
    streams at full rate unlike K=3) with 2-way PE row tiling.
  - The |5S| pass is split between ACT (Abs activation) and DVE (zneg=-5S
    then max(zneg,-zneg), both-SBUF so walrus accepts it) to balance the two
    bottleneck engines; the row max rides along on whichever source is local.
  - 2 of 8 output i-tiles ride along with stage-2 (accumulating in dedicated
    PSUM banks as each e2 j-tile appears); the remaining 6 run as pairs from
    SBUF right after, overlapping the other batch's work.
"""

import numpy as np
import ml_dtypes

import concourse.bass as bass
import concourse.bacc as bacc
import concourse.tile as tile
import concourse.mybir as mybir
from concourse.bass_utils import run_bass_kernel_spmd

B, C, H, W = 16, 512, 32, 32
HW = H * W          # 1024
NCORES = 8
BPC = B // NCORES   # batches per core
NT = HW // 128      # 128-row tiles per HW dim
F32 = mybir.dt.float32
BF16 = mybir.dt.bfloat16
AF = mybir.ActivationFunctionType
AX = mybir.AxisListType
ALU = mybir.AluOpType

N_RIDE = 2          # i-tiles riding along with stage-2
# (b, ti) whose |5S| runs on DVE (2 ops) instead of ACT, balancing engines
DVE_ABS = {(1, 1), (1, 3), (1, 5)}


# ---------------------------------------------------------------- host math
def _line_coeffs(K1, K2, R, t):
    """Float32 numpy mirror of the reference's per-batch line geometry.

    Returns Q (B, 3, HW) with rows [alpha, beta, gamma] and P (3, HW) with
    rows [px, py, 1].
    """
    K1 = np.asarray(K1, np.float32)
    K2 = np.asarray(K2, np.float32)
    R = np.asarray(R, np.float32)
    t = np.asarray(t, np.float32)

    z = np.zeros_like(t[:, 0])
    tx, ty, tz = t[:, 0], t[:, 1], t[:, 2]
    skew = np.stack(
        [
            np.stack([z, -tz, ty], axis=-1),
            np.stack([tz, z, -tx], axis=-1),
            np.stack([-ty, tx, z], axis=-1),
        ],
        axis=1,
    )
    E = skew @ R
    U, S, Vt = np.linalg.svd(E)
    S = S * np.array([1.0, 1.0, 0.0], dtype=S.dtype)
    E = U @ (S[:, :, None] * Vt)
    Fm = np.linalg.inv(np.swapaxes(K2, 1, 2)) @ E @ np.linalg.inv(K1)
    Fm = Fm.astype(np.float32)

    ix, iy = np.meshgrid(
        np.arange(H, dtype=np.float32), np.arange(W, dtype=np.float32), indexing="ij"
    )
    px = ix.reshape(-1)
    py = iy.reshape(-1)
    idx = np.stack([px, py, np.ones_like(px)], axis=0)  # (3, HW)

    lines = Fm @ idx[None]  # (B, 3, HW)
    a, b, c = lines[:, 0], lines[:, 1], lines[:, 2]
    x0 = np.zeros_like(a)
    y0 = -c / b
    x1 = np.full_like(a, float(W))
    y1 = -(c + a * float(W)) / b
    dx = x0 - x1
    dy = y0 - y1
    L = np.sqrt(dx * dx + dy * dy)

    alpha = dy / L
    beta = -dx / L
    gamma = (y0 * dx) / L
    Q = np.stack([alpha, beta, gamma], axis=1).astype(np.float32)  # (B, 3, HW)
    P = idx.astype(np.float32)
    return Q, P


# ---------------------------------------------------------------- device IR
def _build_nc():
    nc = bacc.Bacc("TRN2", target_bir_lowering=False, debug=False)

    # P6/Q6 carry [hi; lo] bf16 splits stacked to K=6, replicated at partition
    # offsets 0/32 so 2-way PE row tiling can run both nh matmuls at once.
    pmat_d = nc.dram_tensor("pmat", [128, HW], BF16, kind="ExternalInput")
    qmat_d = nc.dram_tensor("qmat", [BPC, 128, HW], BF16, kind="ExternalInput")
    fsrc_d = nc.dram_tensor("fsrc", [BPC, HW, C], BF16, kind="ExternalInput")
    ident_d = nc.dram_tensor("ident", [128, 128], BF16, kind="ExternalInput")
    out_d = nc.dram_tensor("out", [BPC, HW, C], F32, kind="ExternalOutput")

    with tile.TileContext(nc) as tc:
        with (
            tc.tile_pool(name="const", bufs=1) as const,
            tc.tile_pool(name="q", bufs=2) as qpool,
            tc.tile_pool(name="f", bufs=2) as fpool,
            tc.tile_pool(name="z", bufs=3) as zpool,
            tc.tile_pool(name="zn", bufs=2) as znpool,
            tc.tile_pool(name="e", bufs=2) as epool,
            tc.tile_pool(name="dg", bufs=2) as dgpool,
            tc.tile_pool(name="e2", bufs=2) as e2pool,
            tc.tile_pool(name="stat", bufs=2) as stat,
            tc.tile_pool(name="o", bufs=4) as opool,
            tc.tile_pool(name="sps", bufs=2, space="PSUM") as spspool,
            tc.tile_pool(name="tp", bufs=1, space="PSUM") as tppool,
            tc.tile_pool(name="g", bufs=1, space="PSUM") as gpool,
        ):
            pm = const.tile([128, HW], BF16)
            idn = const.tile([128, 128], BF16)
            # split by partition halves so two queues carry each tensor
            for ph in range(2):
                nc.sync.dma_start(
                    pm[ph * 64 : (ph + 1) * 64, :], pmat_d[ph * 64 : (ph + 1) * 64, :]
                )
            nc.sync.dma_start(idn[:], ident_d[:])

            st = [dict() for _ in range(BPC)]

            def load_q(b):
                s = st[b]
                s["q"] = qpool.tile([128, HW], BF16, tag="q", name="q")
                for ph in range(2):
                    nc.sync.dma_start(
                        s["q"][ph * 64 : (ph + 1) * 64, :],
                        qmat_d[b, ph * 64 : (ph + 1) * 64, :],
                    )

            def load_rest(b):
                s = st[b]
                s["fa"] = fpool.tile([128, NT, C], BF16, tag="fa", name="fa")
                for tj in range(NT):
                    nc.sync.dma_start(
                        s["fa"][:, tj, :], fsrc_d[b, tj * 128 : (tj + 1) * 128, :]
                    )
                s["ea"] = epool.tile([128, NT, HW], BF16, tag="ea", name="ea")
                s["ms"] = stat.tile([128, NT], F32, tag="ms", name="ms")
                s["s1"] = stat.tile([128, NT], F32, tag="s1", name="s1")
                s["r1"] = stat.tile([128, NT], F32, tag="r1", name="r1")
                s["dga"] = dgpool.tile([128, NT, 128], BF16, tag="dga", name="dga")
                s["e2"] = e2pool.tile([128, NT, HW], BF16, tag="e2", name="e2")
                s["s2h"] = stat.tile([128, NT, 2], F32, tag="s2h", name="s2h")
                s["tp"] = tppool.tile([128, HW], F32, tag="tp", name="tp")
                s["s2"] = stat.tile([128, NT], F32, tag="s2", name="s2")
                s["r2"] = stat.tile([128, NT], F32, tag="r2", name="r2")

            def stage1(b, ti):
                # S = P^T Q as one K=6 (hi+lo) matmul per 512-col half; the
                # four in-flight matmuls (2 sp bufs x 2 halves) each use their
                # own PE row-tile and PSUM bank, so they stream concurrently.
                s = st[b]
                k = b * NT + ti
                sp = spspool.tile([128, HW], F32, tag="sp")
                for nh in range(2):
                    g = 2 * (k % 2) + nh
                    nc.tensor.matmul(
                        sp[:, nh * 512 : (nh + 1) * 512],
                        pm[32 * g : 32 * g + 6, ti * 128 : (ti + 1) * 128],
                        s["q"][32 * g : 32 * g + 6, nh * 512 : (nh + 1) * 512],
                        start=True,
                        stop=True,
                        tile_position=(32 * g, 0),
                    )
                zt = zpool.tile([128, HW], F32)
                if (b, ti) in DVE_ABS:
                    # DVE path: zn = -sp off PSUM (per half, chasing each
                    # matmul), zt = max(-zn, zn) = |sp|, row max on |zn|
                    # (both-SBUF stt, walrus-legal)
                    zn = znpool.tile([128, HW], F32)
                    for nh in range(2):
                        nc.vector.tensor_scalar_mul(
                            zn[:, nh * 512 : (nh + 1) * 512],
                            sp[:, nh * 512 : (nh + 1) * 512], -1.0,
                        )
                    nc.vector.reduce_max(
                        s["ms"][:, ti : ti + 1], zn[:], axis=AX.X,
                        apply_absolute_value=True, negate=True,
                    )
                    nc.vector.scalar_tensor_tensor(
                        zt[:], zn[:], -1.0, zn[:], op0=ALU.mult, op1=ALU.max
                    )
                else:
                    # ACT path: zt = |sp| on ACT; row max straight off PSUM
                    nc.vector.reduce_max(
                        s["ms"][:, ti : ti + 1], sp[:], axis=AX.X,
                        apply_absolute_value=True, negate=True,
                    )
                    nc.scalar.activation(zt[:], sp[:], AF.Abs)
                # e = exp(|5S| - max|5S|), s1 = row sums (sp is 5S already)
                nc.scalar.activation(
                    s["ea"][:, ti, :],
                    zt[:],
                    AF.Exp,
                    bias=s["ms"][:, ti : ti + 1],
                    accum_out=s["s1"][:, ti : ti + 1],
                )

            def recip_dga(b, quarter):
                # batched r1 = 1/s1 for 2 tiles, then diag(r1) prep
                s = st[b]
                lo = quarter * 2
                nc.vector.reciprocal(
                    s["r1"][:, lo : lo + 2], s["s1"][:, lo : lo + 2]
                )
                for ti in range(lo, lo + 2):
                    nc.vector.tensor_scalar_mul(
                        s["dga"][:, ti, :], idn[:], s["r1"][:, ti : ti + 1]
                    )

            def stage2(b, tj):
                # "transpose" via real matmul: PT[j,i'] = sum_i e[i,j]*dg[i,i']
                # = e[i',j]/s1[i'];  E2 = exp(-p) with column sums; fold 1/s2
                # into the f rows.
                s = st[b]
                tp = s["tp"]
                # batch 0's exp runs full width (its ACT phase is dense with
                # stage-1 filler); batch 1's runs as halves so the next tile's
                # left-half transposes overlap the right-half exp (no ACT
                # filler is available in that phase).
                halved = b == 1
                order = (0, 4, 1, 2, 3, 5, 6, 7) if halved else (0, 4, 1, 5, 2, 6, 3, 7)
                for ti in order:
                    nc.tensor.matmul(
                        tp[:, ti * 128 : (ti + 1) * 128],
                        s["ea"][:, ti, tj * 128 : (tj + 1) * 128],
                        s["dga"][:, ti, :],
                        start=True,
                        stop=True,
                    )
                if halved:
                    for h in range(2):
                        nc.scalar.activation(
                            s["e2"][:, tj, h * 512 : (h + 1) * 512],
                            tp[:, h * 512 : (h + 1) * 512],
                            AF.Exp,
                            scale=-1.0,
                            accum_out=s["s2h"][:, tj, h : h + 1],
                        )
                    nc.vector.scalar_tensor_tensor(
                        s["s2"][:, tj : tj + 1],
                        s["s2h"][:, tj, 0:1], 1.0, s["s2h"][:, tj, 1:2],
                        op0=ALU.mult, op1=ALU.add,
                    )
                else:
                    nc.scalar.activation(
                        s["e2"][:, tj, :],
                        tp[:],
                        AF.Exp,
                        scale=-1.0,
                        accum_out=s["s2"][:, tj : tj + 1],
                    )
                nc.vector.reciprocal(
                    s["r2"][:, tj : tj + 1], s["s2"][:, tj : tj + 1]
                )
                nc.vector.tensor_scalar_mul(
                    s["fa"][:, tj, :], s["fa"][:, tj, :], s["r2"][:, tj : tj + 1]
                )

            def ride_alloc(b):
                s = st[b]
                s["gacc"] = [
                    gpool.tile([128, C], F32, tag=f"g{ig}", name=f"gacc{ig}")
                    for ig in range(N_RIDE)
                ]

            def ride_gemm(b, tj):
                # accumulate the first N_RIDE i-tiles as e2/fw j-tiles appear
                s = st[b]
                for ig in range(N_RIDE):
                    nc.tensor.matmul(
                        s["gacc"][ig][:],
                        s["e2"][:, tj, ig * 128 : (ig + 1) * 128],
                        s["fa"][:, tj, :],
                        start=(tj == 0),
                        stop=(tj == NT - 1),
                    )

            def ride_evict(b, on_act):
                s = st[b]
                cp = nc.scalar.copy if on_act else nc.vector.tensor_copy
                for ig in range(N_RIDE):
                    ost = opool.tile([128, C], F32, tag=f"os{ig}")
                    cp(ost[:], s["gacc"][ig][:])
                    for ph in range(4):
                        nc.sync.dma_start(
                            out_d[b, ig * 128 + ph * 32 : ig * 128 + (ph + 1) * 32, :],
                            ost[ph * 32 : (ph + 1) * 32, :],
                        )

            def post_gemm(b, pair, on_act):
                # i-tile pairs (2,3),(4,5),(6,7): two i-tiles per 2-bank PSUM
                # slot reusing the stage-1 pool; one evict + DMA per pair.
                s = st[b]
                op_ = spspool.tile([128, 2, C], F32, tag="sp")
                for half in range(2):
                    ti = N_RIDE + 2 * pair + half
                    for tj in range(NT):
                        nc.tensor.matmul(
                            op_[:, half, :],
                            s["e2"][:, tj, ti * 128 : (ti + 1) * 128],
                            s["fa"][:, tj, :],
                            start=(tj == 0),
                            stop=(tj == NT - 1),
                        )
                ost = opool.tile([128, 2, C], F32, tag="op")
                if on_act:
                    nc.scalar.copy(ost[:], op_[:])
                else:
                    nc.vector.tensor_copy(ost[:], op_[:])
                lo = (N_RIDE + 2 * pair) * 128
                for half in range(2):
                    for ph in range(4):
                        nc.sync.dma_start(
                            out_d[b, lo + half * 128 + ph * 32 : lo + half * 128 + (ph + 1) * 32, :],
                            ost[ph * 32 : (ph + 1) * 32, half, :],
                        )

            # ---- schedule ----
            load_q(0)
            load_q(1)
            load_rest(0)
            load_rest(1)
            # phase A: stage-1 of batch 0
            for ti in range(NT):
                stage1(0, ti)
                if ti % 2 == 1 and ti < NT - 1:
                    recip_dga(0, ti // 2)
            recip_dga(0, 3)
            # phase B: stage-1 of batch 1 interleaved with stage-2 + riding
            # GEMM of batch 0
            ride_alloc(0)
            stage1(1, 0)
            stage1(1, 1)
            for k in range(NT):
                stage2(0, k)
                if k >= 1:
                    ride_gemm(0, k - 1)
                if k < NT - 2:
                    stage1(1, k + 2)
                if k % 2 == 1 and k < NT - 1:
                    recip_dga(1, k // 2)
            recip_dga(1, 3)
            ride_gemm(0, NT - 1)
            ride_evict(0, on_act=False)
            # phase C: stage-2 + riding GEMM of batch 1, post GEMM of batch 0
            ride_alloc(1)
            for k in range(NT):
                stage2(1, k)
                if k >= 1:
                    ride_gemm(1, k - 1)
                if k % 3 == 2:
                    post_gemm(0, k // 3, on_act=False)
            ride_gemm(1, NT - 1)
            post_gemm(0, 2, on_act=False)
            # phase D: tail — ACT is idle, use it for evictions
            ride_evict(1, on_act=True)
            for pair in range(3):
                post_gemm(1, pair, on_act=True)
    nc.compile()
    return nc


_NC = None


def _get_nc():
    global _NC
    if _NC is None:
        _NC = _build_nc()
    return _NC


# ---------------------------------------------------------------- execution
def _run(inputs, trace=False):
    f_src = np.asarray(inputs["f_src"], np.float32)
    Q, P = _line_coeffs(inputs["K1"], inputs["K2"], inputs["R"], inputs["t"])

    fsrcT = np.ascontiguousarray(
        f_src.reshape(B, C, HW).transpose(0, 2, 1)
    ).astype(ml_dtypes.bfloat16)
    ident = np.eye(128, dtype=np.float32).astype(ml_dtypes.bfloat16)

    q_hi = Q.astype(ml_dtypes.bfloat16)
    q_lo = (Q - q_hi.astype(np.float32)).astype(ml_dtypes.bfloat16)
    # K=6 stack [hi; lo], replicated at partition offsets 0/32/64/96
    q6 = np.concatenate([q_hi, q_lo], axis=1)  # (B, 6, HW) bf16
    q_rep = np.zeros((B, 128, HW), dtype=ml_dtypes.bfloat16)
    p_rep = np.zeros((128, HW), dtype=ml_dtypes.bfloat16)
    # fold the x5 distance scale into P: 5*px <= 155 stays exact in bf16
    p6 = np.concatenate([5.0 * P, 5.0 * P], axis=0).astype(ml_dtypes.bfloat16)
    for g in range(4):
        q_rep[:, 32 * g : 32 * g + 6, :] = q6
        p_rep[32 * g : 32 * g + 6, :] = p6

    in_maps = []
    for core in range(NCORES):
        lo = core * BPC
        hi = lo + BPC
        in_maps.append(
            {
                "pmat": p_rep,
                "qmat": np.ascontiguousarray(q_rep[lo:hi]),
                "fsrc": np.ascontiguousarray(fsrcT[lo:hi]),
                "ident": ident,
            }
        )

    nc = _get_nc()
    res = run_bass_kernel_spmd(nc, in_maps, list(range(NCORES)), trace=trace)
    out_flat = np.concatenate(
        [res.results[i]["out"] for i in range(NCORES)], axis=0
    )  # (B, HW, C)
    out = np.ascontiguousarray(out_flat).reshape(B, C, H, W)
    return out, res


def kernel(**inputs):
    out, _ = _run(inputs, trace=False)
    return out


# revision 17
# speedup vs baseline: 1.0055x; 1.0055x over previous
"""Epipolar attention kernel for Trainium2 (8 NeuronCores, batch-parallel).

Math notes (derived from the reference):
  - f_tar is dead code: the output only depends on f_src / K1 / K2 / R / t.
  - With x0=0, x1=W the distance field factorizes rank-3:
        d[b,i,j] = |px_i*alpha[b,j] + py_i*beta[b,j] + gamma[b,j]|
    where alpha = dy/L, beta = -dx/L, gamma = y0*dx/L, L = sqrt(dx^2+dy^2).
  - softmax_j(5*(d-0.1)) == softmax_j(5*d)           (shift invariance)
  - softmax_i(1 - p)     == softmax_i(-p), and p in (0,1] means exp(-p) needs
    no max subtraction.
The 3x3 SVD / inverse chain (B=16) plus the rank-3 coefficient prep is O(B*HW)
host work; all O(B*HW^2) work runs on the NeuronCores.

v4 performance structure:
  - Stage-1 S = P^T Q runs as K=6 matmuls (hi+lo bf16 rows merged; K=6
    streams at full rate unlike K=3) with 2-way PE row tiling.
  - The |5S| pass is split between ACT (Abs activation) and DVE (zneg=-5S
    then max(zneg,-zneg), both-SBUF so walrus accepts it) to balance the two
    bottleneck engines; the row max rides along on whichever source is local.
  - 2 of 8 output i-tiles ride along with stage-2 (accumulating in dedicated
    PSUM banks as each e2 j-tile appears); the remaining 6 run as pairs from
    SBUF right after, overlapping the other batch's work.
"""

import numpy as np
import ml_dtypes

import concourse.bass as bass
import concourse.bacc as bacc
import concourse.tile as tile
import concourse.mybir as mybir
from concourse.bass_utils import run_bass_kernel_spmd

B, C, H, W = 16, 512, 32, 32
HW = H * W          # 1024
NCORES = 8
BPC = B // NCORES   # batches per core
NT = HW // 128      # 128-row tiles per HW dim
F32 = mybir.dt.float32
BF16 = mybir.dt.bfloat16
AF = mybir.ActivationFunctionType
AX = mybir.AxisListType
ALU = mybir.AluOpType

N_RIDE = 2          # i-tiles riding along with stage-2
# (b, ti) whose |5S| runs on DVE (2 ops) instead of ACT, balancing engines
DVE_ABS = {(1, 1), (1, 3), (1, 5)}


# ---------------------------------------------------------------- host math
def _line_coeffs(K1, K2, R, t):
    """Float32 numpy mirror of the reference's per-batch line geometry.

    Returns Q (B, 3, HW) with rows [alpha, beta, gamma] and P (3, HW) with
    rows [px, py, 1].
    """
    K1 = np.asarray(K1, np.float32)
    K2 = np.asarray(K2, np.float32)
    R = np.asarray(R, np.float32)
    t = np.asarray(t, np.float32)

    z = np.zeros_like(t[:, 0])
    tx, ty, tz = t[:, 0], t[:, 1], t[:, 2]
    skew = np.stack(
        [
            np.stack([z, -tz, ty], axis=-1),
            np.stack([tz, z, -tx], axis=-1),
            np.stack([-ty, tx, z], axis=-1),
        ],
        axis=1,
    )
    E = skew @ R
    U, S, Vt = np.linalg.svd(E)
    S = S * np.array([1.0, 1.0, 0.0], dtype=S.dtype)
    E = U @ (S[:, :, None] * Vt)
    Fm = np.linalg.inv(np.swapaxes(K2, 1, 2)) @ E @ np.linalg.inv(K1)
    Fm = Fm.astype(np.float32)

    ix, iy = np.meshgrid(
        np.arange(H, dtype=np.float32), np.arange(W, dtype=np.float32), indexing="ij"
    )
    px = ix.reshape(-1)
    py = iy.reshape(-1)
    idx = np.stack([px, py, np.ones_like(px)], axis=0)  # (3, HW)

    lines = Fm @ idx[None]  # (B, 3, HW)
    a, b, c = lines[:, 0], lines[:, 1], lines[:, 2]
    x0 = np.zeros_like(a)
    y0 = -c / b
    x1 = np.full_like(a, float(W))
    y1 = -(c + a * float(W)) / b
    dx = x0 - x1
    dy = y0 - y1
    L = np.sqrt(dx * dx + dy * dy)

    alpha = dy / L
    beta = -dx / L
    gamma = (y0 * dx) / L
    Q = np.stack([alpha, beta, gamma], axis=1).astype(np.float32)  # (B, 3, HW)
    P = idx.astype(np.float32)
    return Q, P


# ---------------------------------------------------------------- device IR
def _build_nc():
    nc = bacc.Bacc("TRN2", target_bir_lowering=False, debug=False)

    # P6/Q6 carry [hi; lo] bf16 splits stacked to K=6, replicated at partition
    # offsets 0/32 so 2-way PE row tiling can run both nh matmuls at once.
    pmat_d = nc.dram_tensor("pmat", [128, HW], BF16, kind="ExternalInput")
    qmat_d = nc.dram_tensor("qmat", [BPC, 128, HW], BF16, kind="ExternalInput")
    fsrc_d = nc.dram_tensor("fsrc", [BPC, HW, C], BF16, kind="ExternalInput")
    ident_d = nc.dram_tensor("ident", [128, 128], BF16, kind="ExternalInput")
    out_d = nc.dram_tensor("out", [BPC, HW, C], F32, kind="ExternalOutput")

    with tile.TileContext(nc) as tc:
        with (
            tc.tile_pool(name="const", bufs=1) as const,
            tc.tile_pool(name="q", bufs=2) as qpool,
            tc.tile_pool(name="f", bufs=2) as fpool,
            tc.tile_pool(name="z", bufs=3) as zpool,
            tc.tile_pool(name="zn", bufs=2) as znpool,
            tc.tile_pool(name="e", bufs=2) as epool,
            tc.tile_pool(name="dg", bufs=2) as dgpool,
            tc.tile_pool(name="e2", bufs=2) as e2pool,
            tc.tile_pool(name="stat", bufs=2) as stat,
            tc.tile_pool(name="o", bufs=4) as opool,
            tc.tile_pool(name="sps", bufs=2, space="PSUM") as spspool,
            tc.tile_pool(name="tp", bufs=1, space="PSUM") as tppool,
            tc.tile_pool(name="g", bufs=1, space="PSUM") as gpool,
        ):
            pm = const.tile([128, HW], BF16)
            idn = const.tile([128, 128], BF16)
            # split by partition halves so two queues carry each tensor
            for ph in range(2):
                nc.sync.dma_start(
                    pm[ph * 64 : (ph + 1) * 64, :], pmat_d[ph * 64 : (ph + 1) * 64, :]
                )
            nc.sync.dma_start(idn[:], ident_d[:])

            st = [dict() for _ in range(BPC)]

            def load_q(b):
                s = st[b]
                s["q"] = qpool.tile([128, HW], BF16, tag="q", name="q")
                for ph in range(2):
                    nc.sync.dma_start(
                        s["q"][ph * 64 : (ph + 1) * 64, :],
                        qmat_d[b, ph * 64 : (ph + 1) * 64, :],
                    )

            def load_rest(b):
                s = st[b]
                s["fa"] = fpool.tile([128, NT, C], BF16, tag="fa", name="fa")
                for tj in range(NT):
                    nc.sync.dma_start(
                        s["fa"][:, tj, :], fsrc_d[b, tj * 128 : (tj + 1) * 128, :]
                    )
                s["ea"] = epool.tile([128, NT, HW], BF16, tag="ea", name="ea")
                s["ms"] = stat.tile([128, NT], F32, tag="ms", name="ms")
                s["s1"] = stat.tile([128, NT], F32, tag="s1", name="s1")
                s["r1"] = stat.tile([128, NT], F32, tag="r1", name="r1")
                s["dga"] = dgpool.tile([128, NT, 128], BF16, tag="dga", name="dga")
                s["e2"] = e2pool.tile([128, NT, HW], BF16, tag="e2", name="e2")
                s["s2h"] = stat.tile([128, NT, 2], F32, tag="s2h", name="s2h")
                s["s2"] = stat.tile([128, NT], F32, tag="s2", name="s2")
                s["r2"] = stat.tile([128, NT], F32, tag="r2", name="r2")

            def stage1(b, ti):
                # S = P^T Q as one K=6 (hi+lo) matmul per 512-col half; the
                # four in-flight matmuls (2 sp bufs x 2 halves) each use their
                # own PE row-tile and PSUM bank, so they stream concurrently.
                s = st[b]
                k = b * NT + ti
                sp = spspool.tile([128, HW], F32, tag="sp")
                for nh in range(2):
                    g = 2 * (k % 2) + nh
                    nc.tensor.matmul(
                        sp[:, nh * 512 : (nh + 1) * 512],
                        pm[32 * g : 32 * g + 6, ti * 128 : (ti + 1) * 128],
                        s["q"][32 * g : 32 * g + 6, nh * 512 : (nh + 1) * 512],
                        start=True,
                        stop=True,
                        tile_position=(32 * g, 0),
                    )
                zt = zpool.tile([128, HW], F32)
                if (b, ti) in DVE_ABS:
                    # DVE path: zn = -sp off PSUM (per half, chasing each
                    # matmul), zt = max(-zn, zn) = |sp|, row max on |zn|
                    # (both-SBUF stt, walrus-legal)
                    zn = znpool.tile([128, HW], F32)
                    for nh in range(2):
                        nc.vector.tensor_scalar_mul(
                            zn[:, nh * 512 : (nh + 1) * 512],
                            sp[:, nh * 512 : (nh + 1) * 512], -1.0,
                        )
                    nc.vector.reduce_max(
                        s["ms"][:, ti : ti + 1], zn[:], axis=AX.X,
                        apply_absolute_value=True, negate=True,
                    )
                    nc.vector.scalar_tensor_tensor(
                        zt[:], zn[:], -1.0, zn[:], op0=ALU.mult, op1=ALU.max
                    )
                else:
                    # ACT path: zt = |sp| on ACT; row max straight off PSUM
                    nc.vector.reduce_max(
                        s["ms"][:, ti : ti + 1], sp[:], axis=AX.X,
                        apply_absolute_value=True, negate=True,
                    )
                    nc.scalar.activation(zt[:], sp[:], AF.Abs)
                # e = exp(|5S| - max|5S|), s1 = row sums (sp is 5S already)
                nc.scalar.activation(
                    s["ea"][:, ti, :],
                    zt[:],
                    AF.Exp,
                    bias=s["ms"][:, ti : ti + 1],
                    accum_out=s["s1"][:, ti : ti + 1],
                )

            def recip_dga(b, quarter):
                # batched r1 = 1/s1 for 2 tiles, then diag(r1) prep
                s = st[b]
                lo = quarter * 2
                nc.vector.reciprocal(
                    s["r1"][:, lo : lo + 2], s["s1"][:, lo : lo + 2]
                )
                for ti in range(lo, lo + 2):
                    nc.vector.tensor_scalar_mul(
                        s["dga"][:, ti, :], idn[:], s["r1"][:, ti : ti + 1]
                    )

            def stage2(b, tj):
                # "transpose" via real matmul: PT[j,i'] = sum_i e[i,j]*dg[i,i']
                # = e[i',j]/s1[i'];  E2 = exp(-p) with column sums; fold 1/s2
                # into the f rows.
                s = st[b]
                tp = tppool.tile([128, HW], F32, tag="tp")
                # batch 0's exp runs full width (its ACT phase is dense with
                # stage-1 filler); batch 1's runs as halves so the next tile's
                # left-half transposes overlap the right-half exp (no ACT
                # filler is available in that phase).
                halved = b == 1
                order = (0, 4, 1, 2, 3, 5, 6, 7) if halved else (0, 4, 1, 5, 2, 6, 3, 7)
                for ti in order:
                    nc.tensor.matmul(
                        tp[:, ti * 128 : (ti + 1) * 128],
                        s["ea"][:, ti, tj * 128 : (tj + 1) * 128],
                        s["dga"][:, ti, :],
                        start=True,
                        stop=True,
                    )
                if halved:
                    for h in range(2):
                        nc.scalar.activation(
                            s["e2"][:, tj, h * 512 : (h + 1) * 512],
                            tp[:, h * 512 : (h + 1) * 512],
                            AF.Exp,
                            scale=-1.0,
                            accum_out=s["s2h"][:, tj, h : h + 1],
                        )
                    nc.vector.scalar_tensor_tensor(
                        s["s2"][:, tj : tj + 1],
                        s["s2h"][:, tj, 0:1], 1.0, s["s2h"][:, tj, 1:2],
                        op0=ALU.mult, op1=ALU.add,
                    )
                else:
                    nc.scalar.activation(
                        s["e2"][:, tj, :],
                        tp[:],
                        AF.Exp,
                        scale=-1.0,
                        accum_out=s["s2"][:, tj : tj + 1],
                    )
                nc.vector.reciprocal(
                    s["r2"][:, tj : tj + 1], s["s2"][:, tj : tj + 1]
                )
                nc.vector.tensor_scalar_mul(
                    s["fa"][:, tj, :], s["fa"][:, tj, :], s["r2"][:, tj : tj + 1]
                )

            def ride_alloc(b):
                s = st[b]
                s["gacc"] = [
                    gpool.tile([128, C], F32, tag=f"g{ig}", name=f"gacc{ig}")
                    for ig in range(N_RIDE)
                ]

            def ride_gemm(b, tj):
                # accumulate the first N_RIDE i-tiles as e2/fw j-tiles appear
                s = st[b]
                for ig in range(N_RIDE):
                    nc.tensor.matmul(
                        s["gacc"][ig][:],
                        s["e2"][:, tj, ig * 128 : (ig + 1) * 128],
                        s["fa"][:, tj, :],
                        start=(tj == 0),
                        stop=(tj == NT - 1),
                    )

            def ride_evict(b, on_act):
                s = st[b]
                cp = nc.scalar.copy if on_act else nc.vector.tensor_copy
                for ig in range(N_RIDE):
                    ost = opool.tile([128, C], F32, tag=f"os{ig}")
                    cp(ost[:], s["gacc"][ig][:])
                    for ph in range(4):
                        nc.sync.dma_start(
                            out_d[b, ig * 128 + ph * 32 : ig * 128 + (ph + 1) * 32, :],
                            ost[ph * 32 : (ph + 1) * 32, :],
                        )

            def post_gemm(b, pair, on_act):
                # i-tile pairs (2,3),(4,5),(6,7): two i-tiles per 2-bank PSUM
                # slot reusing the stage-1 pool; one evict + DMA per pair.
                s = st[b]
                op_ = spspool.tile([128, 2, C], F32, tag="sp")
                for half in range(2):
                    ti = N_RIDE + 2 * pair + half
                    for tj in range(NT):
                        nc.tensor.matmul(
                            op_[:, half, :],
                            s["e2"][:, tj, ti * 128 : (ti + 1) * 128],
                            s["fa"][:, tj, :],
                            start=(tj == 0),
                            stop=(tj == NT - 1),
                        )
                ost = opool.tile([128, 2, C], F32, tag="op")
                if on_act:
                    nc.scalar.copy(ost[:], op_[:])
                else:
                    nc.vector.tensor_copy(ost[:], op_[:])
                lo = (N_RIDE + 2 * pair) * 128
                for half in range(2):
                    for ph in range(4):
                        nc.sync.dma_start(
                            out_d[b, lo + half * 128 + ph * 32 : lo + half * 128 + (ph + 1) * 32, :],
                            ost[ph * 32 : (ph + 1) * 32, half, :],
                        )

            # ---- schedule ----
            load_q(0)
            load_q(1)
            load_rest(0)
            load_rest(1)
            # phase A: stage-1 of batch 0
            for ti in range(NT):
                stage1(0, ti)
                if ti % 2 == 1 and ti < NT - 1:
                    recip_dga(0, ti // 2)
            recip_dga(0, 3)
            # phase B: stage-1 of batch 1 interleaved with stage-2 + riding
            # GEMM of batch 0
            ride_alloc(0)
            stage1(1, 0)
            stage1(1, 1)
            for k in range(NT):
                stage2(0, k)
                if k >= 1:
                    ride_gemm(0, k - 1)
                if k < NT - 2:
                    stage1(1, k + 2)
                if k % 2 == 1 and k < NT - 1:
                    recip_dga(1, k // 2)
            recip_dga(1, 3)
            ride_gemm(0, NT - 1)
            ride_evict(0, on_act=False)
            # phase C: stage-2 + riding GEMM of batch 1, post GEMM of batch 0
            ride_alloc(1)
            for k in range(NT):
                stage2(1, k)
                if k >= 1:
                    ride_gemm(1, k - 1)
                if k % 3 == 2:
                    post_gemm(0, k // 3, on_act=False)
            ride_gemm(1, NT - 1)
            post_gemm(0, 2, on_act=False)
            # phase D: tail — ACT is idle, use it for evictions
            ride_evict(1, on_act=True)
            for pair in range(3):
                post_gemm(1, pair, on_act=True)
    nc.compile()
    return nc


_NC = None


def _get_nc():
    global _NC
    if _NC is None:
        _NC = _build_nc()
    return _NC


# ---------------------------------------------------------------- execution
def _run(inputs, trace=False):
    f_src = np.asarray(inputs["f_src"], np.float32)
    Q, P = _line_coeffs(inputs["K1"], inputs["K2"], inputs["R"], inputs["t"])

    fsrcT = np.ascontiguousarray(
        f_src.reshape(B, C, HW).transpose(0, 2, 1)
    ).astype(ml_dtypes.bfloat16)
    ident = np.eye(128, dtype=np.float32).astype(ml_dtypes.bfloat16)

    q_hi = Q.astype(ml_dtypes.bfloat16)
    q_lo = (Q - q_hi.astype(np.float32)).astype(ml_dtypes.bfloat16)
    # K=6 stack [hi; lo], replicated at partition offsets 0/32/64/96
    q6 = np.concatenate([q_hi, q_lo], axis=1)  # (B, 6, HW) bf16
    q_rep = np.zeros((B, 128, HW), dtype=ml_dtypes.bfloat16)
    p_rep = np.zeros((128, HW), dtype=ml_dtypes.bfloat16)
    # fold the x5 distance scale into P: 5*px <= 155 stays exact in bf16
    p6 = np.concatenate([5.0 * P, 5.0 * P], axis=0).astype(ml_dtypes.bfloat16)
    for g in range(4):
        q_rep[:, 32 * g : 32 * g + 6, :] = q6
        p_rep[32 * g : 32 * g + 6, :] = p6

    in_maps = []
    for core in range(NCORES):
        lo = core * BPC
        hi = lo + BPC
        in_maps.append(
            {
                "pmat": p_rep,
                "qmat": np.ascontiguousarray(q_rep[lo:hi]),
                "fsrc": np.ascontiguousarray(fsrcT[lo:hi]),
                "ident": ident,
            }
        )

    nc = _get_nc()
    res = run_bass_kernel_spmd(nc, in_maps, list(range(NCORES)), trace=trace)
    out_flat = np.concatenate(
        [res.results[i]["out"] for i in range(NCORES)], axis=0
    )  # (B, HW, C)
    out = np.ascontiguousarray(out_flat).reshape(B, C, H, W)
    return out, res


def kernel(**inputs):
    out, _ = _run(inputs, trace=False)
    return out


# revision 18
# speedup vs baseline: 1.1158x; 1.1097x over previous
"""Epipolar attention kernel for Trainium2 (8 NeuronCores, batch-parallel).

Math notes (derived from the reference):
  - f_tar is dead code: the output only depends on f_src / K1 / K2 / R / t.
  - With x0=0, x1=W the distance field factorizes rank-3:
        d[b,i,j] = |px_i*alpha[b,j] + py_i*beta[b,j] + gamma[b,j]|
    where alpha = dy/L, beta = -dx/L, gamma = y0*dx/L, L = sqrt(dx^2+dy^2).
  - softmax_j(5*(d-0.1)) == softmax_j(5*d)           (shift invariance)
  - softmax_i(1 - p)     == softmax_i(-p), and p in (0,1] means exp(-p) needs
    no max subtraction.
The 3x3 SVD / inverse chain (B=16) plus the rank-3 coefficient prep is O(B*HW)
host work; all O(B*HW^2) work runs on the NeuronCores.

v4 performance structure:
  - Stage-1 S = P^T Q runs as K=6 matmuls (hi+lo bf16 rows merged; K=6
    streams at full rate unlike K=3) with 2-way PE row tiling.
  - The |5S| pass is split between ACT (Abs activation) and DVE (zneg=-5S
    then max(zneg,-zneg), both-SBUF so walrus accepts it) to balance the two
    bottleneck engines; the row max rides along on whichever source is local.
  - 2 of 8 output i-tiles ride along with stage-2 (accumulating in dedicated
    PSUM banks as each e2 j-tile appears); the remaining 6 run as pairs from
    SBUF right after, overlapping the other batch's work.
"""

import numpy as np
import ml_dtypes

import concourse.bass as bass
import concourse.bacc as bacc
import concourse.tile as tile
import concourse.mybir as mybir
from concourse.bass_utils import run_bass_kernel_spmd

B, C, H, W = 16, 512, 32, 32
HW = H * W          # 1024
NCORES = 8
BPC = B // NCORES   # batches per core
NT = HW // 128      # 128-row tiles per HW dim
F32 = mybir.dt.float32
BF16 = mybir.dt.bfloat16
AF = mybir.ActivationFunctionType
AX = mybir.AxisListType
ALU = mybir.AluOpType

N_RIDE = 2          # i-tiles riding along with stage-2
# (b, ti) whose |5S| runs on DVE (2 ops) instead of ACT, balancing engines
DVE_ABS = {(1, 1), (1, 3), (1, 5)}


# ---------------------------------------------------------------- host math
def _line_coeffs(K1, K2, R, t):
    """Float32 numpy mirror of the reference's per-batch line geometry.

    Returns Q (B, 3, HW) with rows [alpha, beta, gamma] and P (3, HW) with
    rows [px, py, 1].
    """
    K1 = np.asarray(K1, np.float32)
    K2 = np.asarray(K2, np.float32)
    R = np.asarray(R, np.float32)
    t = np.asarray(t, np.float32)

    z = np.zeros_like(t[:, 0])
    tx, ty, tz = t[:, 0], t[:, 1], t[:, 2]
    skew = np.stack(
        [
            np.stack([z, -tz, ty], axis=-1),
            np.stack([tz, z, -tx], axis=-1),
            np.stack([-ty, tx, z], axis=-1),
        ],
        axis=1,
    )
    E = skew @ R
    U, S, Vt = np.linalg.svd(E)
    S = S * np.array([1.0, 1.0, 0.0], dtype=S.dtype)
    E = U @ (S[:, :, None] * Vt)
    Fm = np.linalg.inv(np.swapaxes(K2, 1, 2)) @ E @ np.linalg.inv(K1)
    Fm = Fm.astype(np.float32)

    ix, iy = np.meshgrid(
        np.arange(H, dtype=np.float32), np.arange(W, dtype=np.float32), indexing="ij"
    )
    px = ix.reshape(-1)
    py = iy.reshape(-1)
    idx = np.stack([px, py, np.ones_like(px)], axis=0)  # (3, HW)

    lines = Fm @ idx[None]  # (B, 3, HW)
    a, b, c = lines[:, 0], lines[:, 1], lines[:, 2]
    x0 = np.zeros_like(a)
    y0 = -c / b
    x1 = np.full_like(a, float(W))
    y1 = -(c + a * float(W)) / b
    dx = x0 - x1
    dy = y0 - y1
    L = np.sqrt(dx * dx + dy * dy)

    alpha = dy / L
    beta = -dx / L
    gamma = (y0 * dx) / L
    Q = np.stack([alpha, beta, gamma], axis=1).astype(np.float32)  # (B, 3, HW)
    P = idx.astype(np.float32)
    return Q, P


# ---------------------------------------------------------------- device IR
def _build_nc():
    nc = bacc.Bacc("TRN2", target_bir_lowering=False, debug=False)

    # P6/Q6 carry [hi; lo] bf16 splits stacked to K=6, replicated at partition
    # offsets 0/32 so 2-way PE row tiling can run both nh matmuls at once.
    pmat_d = nc.dram_tensor("pmat", [128, HW], BF16, kind="ExternalInput")
    qmat_d = nc.dram_tensor("qmat", [BPC, 128, HW], BF16, kind="ExternalInput")
    fsrc_d = nc.dram_tensor("fsrc", [BPC, HW, C], BF16, kind="ExternalInput")
    ident_d = nc.dram_tensor("ident", [128, 128], BF16, kind="ExternalInput")
    out_d = nc.dram_tensor("out", [BPC, HW, C], F32, kind="ExternalOutput")

    with tile.TileContext(nc) as tc:
        with (
            tc.tile_pool(name="const", bufs=1) as const,
            tc.tile_pool(name="q", bufs=2) as qpool,
            tc.tile_pool(name="f", bufs=2) as fpool,
            tc.tile_pool(name="z", bufs=3) as zpool,
            tc.tile_pool(name="zn", bufs=2) as znpool,
            tc.tile_pool(name="e", bufs=2) as epool,
            tc.tile_pool(name="dg", bufs=2) as dgpool,
            tc.tile_pool(name="e2", bufs=2) as e2pool,
            tc.tile_pool(name="stat", bufs=2) as stat,
            tc.tile_pool(name="o", bufs=4) as opool,
            tc.tile_pool(name="sps", bufs=2, space="PSUM") as spspool,
            tc.tile_pool(name="tp", bufs=1, space="PSUM") as tppool,
            tc.tile_pool(name="g", bufs=1, space="PSUM") as gpool,
        ):
            pm = const.tile([128, HW], BF16)
            idn = const.tile([128, 128], BF16)
            # split by partition halves so two queues carry each tensor
            for ph in range(2):
                nc.sync.dma_start(
                    pm[ph * 64 : (ph + 1) * 64, :], pmat_d[ph * 64 : (ph + 1) * 64, :]
                )
            nc.sync.dma_start(idn[:], ident_d[:])

            st = [dict() for _ in range(BPC)]

            def load_q(b):
                s = st[b]
                s["q"] = qpool.tile([128, HW], BF16, tag="q", name="q")
                for ph in range(2):
                    nc.sync.dma_start(
                        s["q"][ph * 64 : (ph + 1) * 64, :],
                        qmat_d[b, ph * 64 : (ph + 1) * 64, :],
                    )

            def load_rest(b):
                s = st[b]
                s["fa"] = fpool.tile([128, NT, C], BF16, tag="fa", name="fa")
                for tj in range(NT):
                    nc.sync.dma_start(
                        s["fa"][:, tj, :], fsrc_d[b, tj * 128 : (tj + 1) * 128, :]
                    )
                s["ea"] = epool.tile([128, NT, HW], BF16, tag="ea", name="ea")
                s["ms"] = stat.tile([128, NT], F32, tag="ms", name="ms")
                s["s1"] = stat.tile([128, NT], F32, tag="s1", name="s1")
                s["r1"] = stat.tile([128, NT], F32, tag="r1", name="r1")
                s["dga"] = dgpool.tile([128, NT, 128], BF16, tag="dga", name="dga")
                s["e2"] = e2pool.tile([128, NT, HW], BF16, tag="e2", name="e2")
                s["s2h"] = stat.tile([128, NT, 2], F32, tag="s2h", name="s2h")
                s["s2"] = stat.tile([128, NT], F32, tag="s2", name="s2")
                s["r2"] = stat.tile([128, NT], F32, tag="r2", name="r2")

            def stage1(b, ti):
                # S = P^T Q as one K=6 (hi+lo) matmul per 512-col half; the
                # four in-flight matmuls (2 sp bufs x 2 halves) each use their
                # own PE row-tile and PSUM bank, so they stream concurrently.
                s = st[b]
                k = b * NT + ti
                sp = spspool.tile([128, HW], F32, tag="sp")
                for nh in range(2):
                    g = 2 * (k % 2) + nh
                    nc.tensor.matmul(
                        sp[:, nh * 512 : (nh + 1) * 512],
                        pm[32 * g : 32 * g + 6, ti * 128 : (ti + 1) * 128],
                        s["q"][32 * g : 32 * g + 6, nh * 512 : (nh + 1) * 512],
                        start=True,
                        stop=True,
                        tile_position=(32 * g, 0),
                    )
                zt = zpool.tile([128, HW], F32)
                if (b, ti) in DVE_ABS:
                    # DVE path: zn = -sp off PSUM (per half, chasing each
                    # matmul), zt = max(-zn, zn) = |sp|, row max on |zn|
                    # (both-SBUF stt, walrus-legal)
                    zn = znpool.tile([128, HW], F32)
                    for nh in range(2):
                        nc.vector.tensor_scalar_mul(
                            zn[:, nh * 512 : (nh + 1) * 512],
                            sp[:, nh * 512 : (nh + 1) * 512], -1.0,
                        )
                    nc.vector.reduce_max(
                        s["ms"][:, ti : ti + 1], zn[:], axis=AX.X,
                        apply_absolute_value=True, negate=True,
                    )
                    nc.vector.scalar_tensor_tensor(
                        zt[:], zn[:], -1.0, zn[:], op0=ALU.mult, op1=ALU.max
                    )
                else:
                    # ACT path: zt = |sp| on ACT; row max straight off PSUM
                    nc.vector.reduce_max(
                        s["ms"][:, ti : ti + 1], sp[:], axis=AX.X,
                        apply_absolute_value=True, negate=True,
                    )
                    nc.scalar.activation(zt[:], sp[:], AF.Abs)
                # e = exp(|5S| - max|5S|), s1 = row sums (sp is 5S already)
                nc.scalar.activation(
                    s["ea"][:, ti, :],
                    zt[:],
                    AF.Exp,
                    bias=s["ms"][:, ti : ti + 1],
                    accum_out=s["s1"][:, ti : ti + 1],
                )

            def recip_dga(b, quarter):
                # batched r1 = 1/s1 for 2 tiles, then diag(r1) prep
                s = st[b]
                lo = quarter * 2
                nc.vector.reciprocal(
                    s["r1"][:, lo : lo + 2], s["s1"][:, lo : lo + 2]
                )
                for ti in range(lo, lo + 2):
                    nc.vector.tensor_scalar_mul(
                        s["dga"][:, ti, :], idn[:], s["r1"][:, ti : ti + 1]
                    )

            def stage2(b, tj):
                # "transpose" via real matmul: PT[j,i'] = sum_i e[i,j]*dg[i,i']
                # = e[i',j]/s1[i'];  E2 = exp(-p) with column sums; fold 1/s2
                # into the f rows.
                s = st[b]
                tp = tppool.tile([128, HW], F32, tag="tp")
                # batch 0's exp runs full width (its ACT phase is dense with
                # stage-1 filler); batch 1's runs as halves so the next tile's
                # left-half transposes overlap the right-half exp (no ACT
                # filler is available in that phase).
                halved = b == 1
                order = (0, 4, 1, 2, 3, 5, 6, 7) if halved else (0, 4, 1, 5, 2, 6, 3, 7)
                for ti in order:
                    nc.tensor.matmul(
                        tp[:, ti * 128 : (ti + 1) * 128],
                        s["ea"][:, ti, tj * 128 : (tj + 1) * 128],
                        s["dga"][:, ti, :],
                        start=True,
                        stop=True,
                    )
                if halved:
                    for h in range(2):
                        nc.scalar.activation(
                            s["e2"][:, tj, h * 512 : (h + 1) * 512],
                            tp[:, h * 512 : (h + 1) * 512],
                            AF.Exp,
                            scale=-1.0,
                            accum_out=s["s2h"][:, tj, h : h + 1],
                        )
                    nc.vector.scalar_tensor_tensor(
                        s["s2"][:, tj : tj + 1],
                        s["s2h"][:, tj, 0:1], 1.0, s["s2h"][:, tj, 1:2],
                        op0=ALU.mult, op1=ALU.add,
                    )
                else:
                    nc.scalar.activation(
                        s["e2"][:, tj, :],
                        tp[:],
                        AF.Exp,
                        scale=-1.0,
                        accum_out=s["s2"][:, tj : tj + 1],
                    )
                nc.vector.reciprocal(
                    s["r2"][:, tj : tj + 1], s["s2"][:, tj : tj + 1]
                )
                nc.vector.tensor_scalar_mul(
                    s["fa"][:, tj, :], s["fa"][:, tj, :], s["r2"][:, tj : tj + 1]
                )

            def ride_alloc(b):
                s = st[b]
                s["gacc"] = [
                    gpool.tile([128, C], F32, tag=f"g{ig}", name=f"gacc{ig}")
                    for ig in range(N_RIDE)
                ]

            def ride_gemm(b, tj):
                # accumulate the first N_RIDE i-tiles as e2/fw j-tiles appear
                s = st[b]
                for ig in range(N_RIDE):
                    nc.tensor.matmul(
                        s["gacc"][ig][:],
                        s["e2"][:, tj, ig * 128 : (ig + 1) * 128],
                        s["fa"][:, tj, :],
                        start=(tj == 0),
                        stop=(tj == NT - 1),
                    )

            def ride_evict(b, on_act):
                s = st[b]
                cp = nc.scalar.copy if on_act else nc.vector.tensor_copy
                for ig in range(N_RIDE):
                    ost = opool.tile([128, C], F32, tag=f"os{ig}")
                    cp(ost[:], s["gacc"][ig][:])
                    for ph in range(2):
                        nc.sync.dma_start(
                            out_d[b, ig * 128 + ph * 64 : ig * 128 + (ph + 1) * 64, :],
                            ost[ph * 64 : (ph + 1) * 64, :],
                        )

            def post_gemm(b, pair, on_act):
                # i-tile pairs (2,3),(4,5),(6,7): two i-tiles per 2-bank PSUM
                # slot reusing the stage-1 pool; one evict + DMA per pair.
                s = st[b]
                op_ = spspool.tile([128, 2, C], F32, tag="sp")
                for half in range(2):
                    ti = N_RIDE + 2 * pair + half
                    for tj in range(NT):
                        nc.tensor.matmul(
                            op_[:, half, :],
                            s["e2"][:, tj, ti * 128 : (ti + 1) * 128],
                            s["fa"][:, tj, :],
                            start=(tj == 0),
                            stop=(tj == NT - 1),
                        )
                ost = opool.tile([128, 2, C], F32, tag="op")
                if on_act:
                    nc.scalar.copy(ost[:], op_[:])
                else:
                    nc.vector.tensor_copy(ost[:], op_[:])
                lo = (N_RIDE + 2 * pair) * 128
                for half in range(2):
                    for ph in range(2):
                        nc.sync.dma_start(
                            out_d[b, lo + half * 128 + ph * 64 : lo + half * 128 + (ph + 1) * 64, :],
                            ost[ph * 64 : (ph + 1) * 64, half, :],
                        )

            # ---- schedule ----
            load_q(0)
            load_q(1)
            load_rest(0)
            load_rest(1)
            # phase A: stage-1 of batch 0
            for ti in range(NT):
                stage1(0, ti)
                if ti % 2 == 1 and ti < NT - 1:
                    recip_dga(0, ti // 2)
            recip_dga(0, 3)
            # phase B: stage-1 of batch 1 interleaved with stage-2 + riding
            # GEMM of batch 0
            ride_alloc(0)
            stage1(1, 0)
            stage1(1, 1)
            for k in range(NT):
                stage2(0, k)
                if k >= 1:
                    ride_gemm(0, k - 1)
                if k < NT - 2:
                    stage1(1, k + 2)
                if k % 2 == 1 and k < NT - 1:
                    recip_dga(1, k // 2)
            recip_dga(1, 3)
            ride_gemm(0, NT - 1)
            ride_evict(0, on_act=False)
            # phase C: stage-2 + riding GEMM of batch 1, post GEMM of batch 0
            ride_alloc(1)
            for k in range(NT):
                stage2(1, k)
                if k >= 1:
                    ride_gemm(1, k - 1)
                if k % 3 == 2:
                    post_gemm(0, k // 3, on_act=False)
            ride_gemm(1, NT - 1)
            post_gemm(0, 2, on_act=False)
            # phase D: tail — ACT is idle, use it for evictions; the small
            # ride evicts go last so the exposed DMA drain is short
            post_gemm(1, 0, on_act=True)
            ride_evict(1, on_act=True)
            post_gemm(1, 1, on_act=True)
            post_gemm(1, 2, on_act=True)
    nc.compile()
    return nc


_NC = None


def _get_nc():
    global _NC
    if _NC is None:
        _NC = _build_nc()
    return _NC


# ---------------------------------------------------------------- execution
def _run(inputs, trace=False):
    f_src = np.asarray(inputs["f_src"], np.float32)
    Q, P = _line_coeffs(inputs["K1"], inputs["K2"], inputs["R"], inputs["t"])

    fsrcT = np.ascontiguousarray(
        f_src.reshape(B, C, HW).transpose(0, 2, 1)
    ).astype(ml_dtypes.bfloat16)
    ident = np.eye(128, dtype=np.float32).astype(ml_dtypes.bfloat16)

    q_hi = Q.astype(ml_dtypes.bfloat16)
    q_lo = (Q - q_hi.astype(np.float32)).astype(ml_dtypes.bfloat16)
    # K=6 stack [hi; lo], replicated at partition offsets 0/32/64/96
    q6 = np.concatenate([q_hi, q_lo], axis=1)  # (B, 6, HW) bf16
    q_rep = np.zeros((B, 128, HW), dtype=ml_dtypes.bfloat16)
    p_rep = np.zeros((128, HW), dtype=ml_dtypes.bfloat16)
    # fold the x5 distance scale into P: 5*px <= 155 stays exact in bf16
    p6 = np.concatenate([5.0 * P, 5.0 * P], axis=0).astype(ml_dtypes.bfloat16)
    for g in range(4):
        q_rep[:, 32 * g : 32 * g + 6, :] = q6
        p_rep[32 * g : 32 * g + 6, :] = p6

    in_maps = []
    for core in range(NCORES):
        lo = core * BPC
        hi = lo + BPC
        in_maps.append(
            {
                "pmat": p_rep,
                "qmat": np.ascontiguousarray(q_rep[lo:hi]),
                "fsrc": np.ascontiguousarray(fsrcT[lo:hi]),
                "ident": ident,
            }
        )

    nc = _get_nc()
    res = run_bass_kernel_spmd(nc, in_maps, list(range(NCORES)), trace=trace)
    out_flat = np.concatenate(
        [res.results[i]["out"] for i in range(NCORES)], axis=0
    )  # (B, HW, C)
    out = np.ascontiguousarray(out_flat).reshape(B, C, H, W)
    return out, res


def kernel(**inputs):
    out, _ = _run(inputs, trace=False)
    return out
